# revision 21
# baseline (speedup 1.0000x reference)
"""GCNConv Trainium2 kernel: out = segment_sum(features[src], dst) @ W + b.

Strategy (8 NeuronCores, graph partitioned by destination node):
  - Host: pack nodes into custom (core, slot) tiles of <=128 nodes balancing
    lo/hi edge counts; per core, MATCH pairs of source rows that are co-used
    by the same slot and PERMUTE that core's private copy of the feature
    table so matched rows sit adjacent.  A paired 512B dma_gather descriptor
    then feeds TWO edges (two matmul chunks), halving SWDGE descriptor
    generation -- the kernel's bottleneck -- for ~52% of edges, and 512B
    descriptors drain at full line rate (256B ones pay a read-modify-write
    penalty).  Unpaired edges use two 256B single-row streams (table halves,
    int16 index range).
  - Device (per core): three gather streams (pair/single-lo/single-hi)
    emitted eagerly in consumption order over all 4 SWDGE queues; per
    (slot, stream) segment one broadcast tensor_tensor builds all one-hot
    chunks at once (bf16, never enters DVE 2-port mode so it cannot stall
    SWDGE desc-gen); per 128-edge chunk one matmul accumulates msgs.T @
    onehot into PSUM, yielding agg.T per node tile; then out.T = W.T @ agg.T
    on the TensorEngine and a fused bias-add on the Scalar engine.
  - Host: scatter per-core tile outputs back to [50000, 128].
"""

import os
import sys

for _p in ("/opt/trn_rl_repo",):
    if _p not in sys.path and os.path.isdir(_p):
        sys.path.insert(0, _p)

import numpy as np
import ml_dtypes

P = 128
N_NODES = 50000
N_EDGES = 640000
D = 128
NCORES = 8
HALF = 25000          # int16 index-range split for single-row streams
NTILE = (N_NODES + P - 1) // P          # 391
NSLOT = (NTILE + NCORES - 1) // NCORES  # 49 node tiles per core
GCHUNK = 16           # single chunks (128 rows) per dma_gather call
GPAIR = 8             # pair chunks (128 x 512B) per dma_gather call
NQUEUES = 4           # SWDGE queues; gather desc-gen contexts run concurrently
GBUFS = 6             # in-flight gather buffers per stream
SINGLE_PACKET = False

BF16 = ml_dtypes.bfloat16


# ---------------------------------------------------------------- host plan

def _pack_nodes(src, dst):
    """Nodes -> (core, slot) tiles of <=128 nodes, steering per-tile lo/hi
    edge sums toward 128-edge chunk boundaries; returns maps per node."""
    d_lo = np.bincount(dst[src < HALF], minlength=N_NODES).astype(np.int64)
    d_hi = np.bincount(dst[src >= HALF], minlength=N_NODES).astype(np.int64)
    d = d_lo + d_hi

    order = np.argsort(-d, kind="stable")
    core_tot = np.zeros(NCORES)
    core_n = np.zeros(NCORES, int)
    core_of = np.empty(N_NODES, int)
    cap = NSLOT * P
    for n in order:
        c = min((c for c in range(NCORES) if core_n[c] < cap),
                key=lambda c: core_tot[c])
        core_of[n] = c
        core_tot[c] += d[n]
        core_n[c] += 1

    SLACK = 3
    lo_tot = np.bincount(core_of, weights=d_lo, minlength=NCORES)
    hi_tot = np.bincount(core_of, weights=d_hi, minlength=NCORES)
    KA = int(np.ceil(lo_tot.max() / P)) + SLACK
    KB = int(np.ceil(hi_tot.max() / P)) + SLACK

    def distribute(K, slots):
        base, extra = K // slots, K - (K // slots) * slots
        return np.array([base + 1] * extra + [base] * (slots - extra))

    A = distribute(KA, NSLOT)
    B = distribute(KB, NSLOT)[::-1]

    node_lists = [[None] * NSLOT for _ in range(NCORES)]
    slot_of = np.empty(N_NODES, int)
    pos_of = np.empty(N_NODES, int)
    for c in range(NCORES):
        nodes = np.where(core_of == c)[0]
        dl = d_lo[nodes].astype(np.float64)
        dh = d_hi[nodes].astype(np.float64)
        alive = np.ones(len(nodes), bool)
        for s in range(NSLOT):
            TL, TH = A[s] * P, B[s] * P
            lo = hi = 0.0
            rn = P
            take = []
            idxs = np.where(alive)[0]
            for _ in range(min(P, int(alive.sum()))):
                idxs = idxs[alive[idxs]]
                if len(idxs) == 0:
                    break
                bl, bh = (TL - lo) / rn, (TH - hi) / rn
                ok = (dl[idxs] <= TL - lo) & (dh[idxs] <= TH - hi)
                cand = idxs[ok] if ok.any() else idxs
                pick = cand[np.argmin(np.abs(dl[cand] - bl) +
                                      np.abs(dh[cand] - bh))]
                take.append(pick)
                alive[pick] = False
                lo += dl[pick]
                hi += dh[pick]
                rn -= 1
                if rn == 0:
                    break
            ns = nodes[np.array(take, int)] if take else np.empty(0, int)
            node_lists[c][s] = ns
            slot_of[ns] = s
            pos_of[ns] = np.arange(len(ns))
    return core_of, slot_of, pos_of, node_lists


CUT = 32768           # single-row stream index range (int16 positive)
TRIM = 64             # pair-chunk remainders below this become singles


def plan(src, dst):
    src = np.asarray(src).astype(np.int64)
    dst = np.asarray(dst).astype(np.int64)
    core_of, slot_of, pos_of, node_lists = _pack_nodes(src, dst)

    # Per core: match source rows co-used by a slot; couples gather 512B
    # pair elements, leftovers gather 256B single rows (all placed below
    # position CUT by the permutation so one int16 stream covers them).
    percore = []  # dicts with couples/singles per slot + perm
    for c in range(NCORES):
        m = core_of[dst] == c
        es, ed = src[m], dst[m]
        sk = slot_of[ed]
        dv = pos_of[ed].astype(np.float32)  # dstl value per edge
        partner = np.full(N_NODES, -1, np.int64)
        slot_couples = [None] * NSLOT   # (row_even, row_odd, dstlA, dstlB)
        slot_singles = [None] * NSLOT   # (row, dstl)
        pair_order = []                 # (a, b) in formation order, a < b
        for s in range(NSLOT):
            ms = sk == s
            rows_e, dstl_e = es[ms], dv[ms]
            o = np.argsort(rows_e, kind="stable")
            rows_e, dstl_e = rows_e[o], dstl_e[o]
            urows, starts, cnts = np.unique(rows_e, return_index=True,
                                            return_counts=True)
            cnt = dict(zip(urows.tolist(), cnts.tolist()))
            sidx = dict(zip(urows.tolist(), starts.tolist()))
            inslot = set(urows.tolist())
            unm = [r for r in urows.tolist() if partner[r] < 0]
            unm.sort(key=lambda r: -cnt[r])
            for a, b in zip(unm[0::2], unm[1::2]):
                a, b = (a, b) if a < b else (b, a)
                partner[a] = b
                partner[b] = a
                pair_order.append((a, b))
            used = dict.fromkeys(urows.tolist(), 0)
            cps = []
            for a in urows.tolist():
                b = int(partner[a])
                if b < 0 or a >= b or b not in inslot:
                    continue
                k = min(cnt[a], cnt[b])
                for j in range(k):
                    cps.append((a, b, dstl_e[sidx[a] + j], dstl_e[sidx[b] + j]))
                used[a] = used[b] = k
            sg = []
            for r in urows.tolist():
                for j in range(used[r], cnt[r]):
                    sg.append((r, dstl_e[sidx[r] + j]))
            # trim small pair-chunk remainders into singles
            rem = len(cps) % P
            if rem and rem < TRIM:
                for a, b, da, db in cps[-rem:]:
                    sg.append((a, da))
                    sg.append((b, db))
                cps = cps[:-rem]
            slot_couples[s] = cps
            slot_singles[s] = sg
        # permutation: single-needing rows first (below CUT), pairs aligned
        srows = {r for s in range(NSLOT) for r, _ in slot_singles[s]}
        pairs_w = [(a, b) for a, b in pair_order if a in srows or b in srows]
        pairs_wo = [(a, b) for a, b in pair_order
                    if a not in srows and b not in srows]
        unm_single = sorted(r for r in srows if partner[r] < 0)
        assert 2 * len(pairs_w) + len(unm_single) <= CUT, "singles exceed CUT"
        perm = np.empty(N_NODES, np.int64)
        placed = np.zeros(N_NODES, bool)
        p = 0
        for a, b in pairs_w:
            perm[p], perm[p + 1] = a, b
            placed[a] = placed[b] = True
            p += 2
        for r in unm_single:
            perm[p] = r
            placed[r] = True
            p += 1
        if p % 2:  # keep remaining pairs even-aligned
            filler = int(np.where(~placed)[0][0])
            perm[p] = filler
            placed[filler] = True
            p += 1
        for a, b in pairs_wo:
            perm[p], perm[p + 1] = a, b
            placed[a] = placed[b] = True
            p += 2
        rest = np.where(~placed)[0]
        perm[p:] = rest
        posn = np.empty(N_NODES, np.int64)
        posn[perm] = np.arange(N_NODES)
        percore.append({"couples": slot_couples, "singles": slot_singles,
                        "perm": perm, "posn": posn})

    # shared per-slot chunk schedule (max over cores)
    KP = np.zeros(NSLOT, int)
    KSG = np.zeros(NSLOT, int)
    for c in range(NCORES):
        pc = percore[c]
        for s in range(NSLOT):
            KP[s] = max(KP[s], -(-len(pc["couples"][s]) // P))
            KSG[s] = max(KSG[s], -(-len(pc["singles"][s]) // P))
    for s in range(NSLOT):
        if KP[s] + KSG[s] == 0:
            KSG[s] = 1
    KPt, KSGt = int(KP.sum()), int(KSG.sum())
    pbase = np.concatenate([[0], np.cumsum(KP)])[:NSLOT]
    sgbase = np.concatenate([[0], np.cumsum(KSG)])[:NSLOT]
    NCH = 2 * KPt + KSGt                 # matmul chunks
    NGX = KPt + KSGt                     # gidx chunks (128 idxs each)

    # per-core gather index / dstl arrays
    gidx = np.zeros((NCORES, NGX, P), np.int16)
    dstl = np.full((NCORES, NCH, P), -1.0, np.float32)
    for c in range(NCORES):
        pc = percore[c]
        posn = pc["posn"]
        for s in range(NSLOT):
            for j, (a, b, da, db) in enumerate(pc["couples"][s]):
                g, lane = j // P, j % P
                gidx[c, pbase[s] + g, lane] = posn[a] // 2
                dstl[c, 2 * (pbase[s] + g), lane] = da
                dstl[c, 2 * (pbase[s] + g) + 1, lane] = db
            for j, (r, dd) in enumerate(pc["singles"][s]):
                g, lane = j // P, j % P
                assert posn[r] < CUT
                gidx[c, KPt + sgbase[s] + g, lane] = posn[r]
                dstl[c, 2 * KPt + sgbase[s] + g, lane] = dd
    return {
        "node_lists": node_lists,
        "percore": percore,
        "KP": KP, "KSG": KSG,
        "gidx": gidx, "dstl": dstl,
    }


def _groups(K, G):
    out = []
    c = 0
    while c < K:
        out.append((c, min(c + G, K)))
        c = out[-1][1]
    return out


STREAMS = ("pr", "sg")


def _schedules(KP, KSG):
    """Per-stream gather groups + consumption-ordered gather list.
    Units are gidx chunks (pair chunks for pr, single chunks for sg)."""
    groups = {"pr": _groups(int(KP.sum()), GPAIR),
              "sg": _groups(int(KSG.sum()), GCHUNK)}
    per_slot = {"pr": KP, "sg": KSG}
    order = []
    nxt = dict.fromkeys(STREAMS, 0)
    cur = dict.fromkeys(STREAMS, 0)
    for s in range(NSLOT):
        for name in STREAMS:
            cur[name] += int(per_slot[name][s])
            while nxt[name] < len(groups[name]) and \
                    groups[name][nxt[name]][0] < cur[name]:
                order.append((name, groups[name][nxt[name]]))
                nxt[name] += 1
    return groups, order


def pack_gidx(gidx_chunks, groups_all):
    """[NGX,128] int16 chunk-major indices -> [128, NGX*8] dma_gather layout
    (per gather group: idx i at [i%16, i//16], replicated to 128 parts)."""
    NGX = gidx_chunks.shape[0]
    out = np.zeros((128, NGX * 8), np.int16)
    for c0, c1 in groups_all:
        g = gidx_chunks[c0:c1].reshape(-1)
        blk = g.reshape(-1, 16).T
        out[:, c0 * 8:c1 * 8] = np.tile(blk, (8, 1))
    return out


# ---------------------------------------------------------------- program

def build(KP, KSG, dbg=False):
    import concourse.bass as bass
    import concourse.mybir as mybir
    from concourse import bacc
    import concourse.tile as tile

    KPt, KSGt = int(KP.sum()), int(KSG.sum())
    NCH = 2 * KPt + KSGt
    NGX = KPt + KSGt
    bf16, f32, i16 = mybir.dt.bfloat16, mybir.dt.float32, mybir.dt.int16

    pbase = np.concatenate([[0], np.cumsum(KP)])[:NSLOT]
    sgbase = np.concatenate([[0], np.cumsum(KSG)])[:NSLOT]

    nc = bacc.Bacc("TRN2", debug=dbg, num_swdge_queues=NQUEUES)
    fall = nc.dram_tensor("fall", [N_NODES, D], bf16, kind="ExternalInput")
    gidx = nc.dram_tensor("gidx", [P, NGX * 8], i16, kind="ExternalInput")
    dstl = nc.dram_tensor("dstl", [P, NCH], bf16, kind="ExternalInput")
    iota = nc.dram_tensor("iota", [P, P], bf16, kind="ExternalInput")
    wmat = nc.dram_tensor("wmat", [P, P], bf16, kind="ExternalInput")
    bcol = nc.dram_tensor("bcol", [P, 1], f32, kind="ExternalInput")
    out = nc.dram_tensor("out", [P, NSLOT * P], f32, kind="ExternalOutput")

    groups, gorder = _schedules(KP, KSG)
    # gidx-chunk offset and element size (bf16 elements) per stream
    gxoff = {"pr": 0, "sg": KPt}
    elem = {"pr": 2 * P, "sg": P}
    KSEG = int(max((2 * KP).max(), KSG.max(), 1))

    with tile.TileContext(nc) as tc:
        with tc.tile_pool(name="const", bufs=1) as cp, \
             tc.tile_pool(name="gpr", bufs=GBUFS) as gpr, \
             tc.tile_pool(name="gsg", bufs=GBUFS) as gsg, \
             tc.tile_pool(name="oh", bufs=4) as ohp, \
             tc.tile_pool(name="res", bufs=3) as resp, \
             tc.tile_pool(name="psA", bufs=3, space="PSUM") as psA, \
             tc.tile_pool(name="psB", bufs=2, space="PSUM") as psB:

            tabs = {
                "pr": fall[:].rearrange("(n two) d -> n (two d)", two=2),
                "sg": fall[0:CUT, :],
            }
            pools = {"pr": gpr, "sg": gsg}

            # gather indices for the first groups land first so the first
            # gathers can issue immediately; the rest loads behind them
            gidx_t = cp.tile([P, NGX * 8], i16)
            first = gorder[:2]
            done = dict.fromkeys(STREAMS, 0)
            for name, (c0, c1) in first:
                o = gxoff[name]
                nc.sync.dma_start(out=gidx_t[:, (o + c0) * 8:(o + c1) * 8],
                                  in_=gidx[:, (o + c0) * 8:(o + c1) * 8])
                done[name] = max(done[name], c1)

            st = {name: {"tiles": {}, "g": 0} for name in STREAMS}

            def fetch(name):
                S = st[name]
                gi = S["g"]
                c0, c1 = groups[name][gi]
                n = c1 - c0
                e = elem[name]
                t = pools[name].tile([P, n * e], mybir.dt.bfloat16,
                                     tag="g" + name)
                o = gxoff[name]
                nc.gpsimd.dma_gather(
                    out_ap=t[:].rearrange("p (g d) -> p g d", d=e),
                    in_ap=tabs[name],
                    idxs_ap=gidx_t[:, (o + c0) * 8:(o + c1) * 8],
                    num_idxs=n * P,
                    num_idxs_reg=n * P,
                    elem_size=e,
                    single_packet=SINGLE_PACKET,
                )
                S["tiles"][gi] = (t, c0, c1)
                S["g"] += 1

            for name, _ in first:
                fetch(name)

            # remaining gidx in bulk (one tail DMA per stream), then consts
            for name in STREAMS:
                o = gxoff[name]
                c0 = done[name]
                K = {"pr": KPt, "sl": KSLt, "sh": KSHt}[name]
                if c0 < K:
                    nc.sync.dma_start(out=gidx_t[:, (o + c0) * 8:(o + K) * 8],
                                      in_=gidx[:, (o + c0) * 8:(o + K) * 8])
            dstl_t = cp.tile([P, NCH], bf16)
            nc.sync.dma_start(out=dstl_t[:], in_=dstl[:])
            iota_t = cp.tile([P, P], bf16)
            nc.sync.dma_start(out=iota_t[:], in_=iota[:])
            w_t = cp.tile([P, P], bf16)
            nc.sync.dma_start(out=w_t[:], in_=wmat[:])
            b_t = cp.tile([P, 1], f32)
            nc.sync.dma_start(out=b_t[:], in_=bcol[:])

            for name, _ in gorder[2:]:
                fetch(name)

            def onehot(dc0, k):
                oh = ohp.tile([P, KSEG * P], mybir.dt.bfloat16, tag="oh")
                in0 = iota_t[:].rearrange("p (k f) -> p k f", k=1) \
                    .broadcast_to([P, k, P])
                in1 = dstl_t[:, dc0:dc0 + k].rearrange("p (k o) -> p k o", o=1) \
                    .broadcast_to([P, k, P])
                outv = oh[:, :k * P].rearrange("p (k f) -> p k f", k=k)
                nc.vector.tensor_tensor(out=outv, in0=in0, in1=in1,
                                        op=mybir.AluOpType.is_equal)
                return oh

            cur = dict.fromkeys(STREAMS, 0)   # gidx-chunk cursor per stream
            gcur = dict.fromkeys(STREAMS, 0)  # current group idx per stream
            for s in range(NSLOT):
                nch = int(2 * KP[s] + KSL[s] + KSH[s])
                ps_agg = psA.tile([P, P], f32, tag="agg")
                ci = 0
                for name, k, dc0 in (
                    ("pr", int(KP[s]), 2 * int(pbase[s])),
                    ("sl", int(KSL[s]), 2 * KPt + int(slbase[s])),
                    ("sh", int(KSH[s]), 2 * KPt + KSLt + int(shbase[s])),
                ):
                    if k == 0:
                        continue
                    S = st[name]
                    nmm = 2 * k if name == "pr" else k
                    oh = onehot(dc0, nmm)
                    for j in range(k):
                        pos = cur[name]
                        while pos >= groups[name][gcur[name]][1]:
                            gcur[name] += 1
                        t, c0, c1 = S["tiles"][gcur[name]]
                        off = pos - c0
                        e = elem[name]
                        sides = (0, 1) if name == "pr" else (0,)
                        for h in sides:
                            nc.tensor.matmul(
                                out=ps_agg[:],
                                lhsT=t[:, off * e + h * P:off * e + (h + 1) * P],
                                rhs=oh[:, (len(sides) * j + h) * P:
                                       (len(sides) * j + h + 1) * P],
                                start=(ci == 0), stop=(ci == nch - 1),
                            )
                            ci += 1
                        cur[name] += 1

                aggT = resp.tile([P, P], mybir.dt.bfloat16, tag="aggT")
                nc.scalar.copy(out=aggT[:], in_=ps_agg[:])
                ps_out = psB.tile([P, P], f32, tag="out")
                nc.tensor.matmul(out=ps_out[:], lhsT=w_t[:], rhs=aggT[:],
                                 start=True, stop=True)
                o_sb = resp.tile([P, P], f32, tag="osb")
                nc.scalar.activation(
                    out=o_sb[:], in_=ps_out[:],
                    func=mybir.ActivationFunctionType.Identity,
                    bias=b_t[:, 0:1],
                )
                nc.sync.dma_start(out=out[:, s * P:(s + 1) * P], in_=o_sb[:])

    # Spread gathers across SWDGE queues.  Tile assigns each Pool-engine DMA
    # a DMASW completion lane in *scheduled* order; queue choice must be a
    # function of that lane (the sim/ucode bind each lane to one queue), so
    # retag after scheduling: queue = lane % NQUEUES.
    for inst in nc.inst_map.values():
        if isinstance(inst, mybir.InstDMAGatherAnt):
            proc = inst.bass_scheduled_proc
            if proc is not None and 11 <= proc <= 18:
                inst.queue_num = (proc - 11) % NQUEUES

    nc.compile()
    return nc


# ---------------------------------------------------------------- in_maps

def make_in_maps(features, W, b, pl):
    f16 = np.ascontiguousarray(features).astype(BF16)
    iota_np = np.tile(np.arange(P, dtype=np.float32)[None, :], (P, 1)).astype(BF16)
    w_np = np.asarray(W, np.float32).astype(BF16)
    b_np = np.asarray(b, np.float32).reshape(1, D).T.copy()  # [128,1]
    KP, KSL, KSH = pl["KP"], pl["KSL"], pl["KSH"]
    groups = {"pr": _groups(int(KP.sum()), GPAIR),
              "sl": _groups(int(KSL.sum()), GCHUNK),
              "sh": _groups(int(KSH.sum()), GCHUNK)}
    KPt, KSLt = int(KP.sum()), int(KSL.sum())
    all_groups = ([(a, b) for a, b in groups["pr"]]
                  + [(KPt + a, KPt + b) for a, b in groups["sl"]]
                  + [(KPt + KSLt + a, KPt + KSLt + b) for a, b in groups["sh"]])
    in_maps = []
    for c in range(NCORES):
        perm = pl["percore"][c]["perm"]
        in_maps.append({
            "fall": f16[perm],
            "gidx": pack_gidx(pl["gidx"][c], all_groups),
            "dstl": np.ascontiguousarray(pl["dstl"][c].T).astype(BF16),
            "iota": iota_np,
            "wmat": w_np,
            "bcol": b_np,
        })
    return in_maps


def unshard(outs, node_lists):
    full = np.zeros((N_NODES, D), np.float32)
    for c in range(NCORES):
        oT = np.asarray(outs[c]["out"], np.float32)
        for s in range(NSLOT):
            ns = node_lists[c][s]
            if len(ns) == 0:
                continue
            full[ns, :] = oT[:, s * P:s * P + len(ns)].T
    return full


# ---------------------------------------------------------------- entry

_CACHE = {}


def kernel(features, src, dst, W, b):
    from concourse.bass_utils import run_bass_kernel_spmd

    pl = plan(src, dst)
    key = (tuple(pl["KP"]), tuple(pl["KSL"]), tuple(pl["KSH"]))
    if key not in _CACHE:
        _CACHE[key] = build(pl["KP"], pl["KSL"], pl["KSH"])
    nc = _CACHE[key]
    in_maps = make_in_maps(features, W, b, pl)
    last = None
    for _ in range(3):  # retry: a previously wedged pool device can fail a load
        try:
            res = run_bass_kernel_spmd(nc, in_maps, core_ids=list(range(NCORES)))
            return unshard(res.results, pl["node_lists"])
        except Exception as e:  # noqa: BLE001
            last = e
    raise last


# revision 22
# speedup vs baseline: 1.2817x; 1.2817x over previous
"""GCNConv Trainium2 kernel: out = segment_sum(features[src], dst) @ W + b.

Strategy (8 NeuronCores, graph partitioned by destination node):
  - Host: partition the 391 dst-node tiles (128 nodes each) across 8 cores
    (LPT balance by edge count).  Edges live with their dst tile.  Features
    are replicated to every core in bf16, split into two 25000-row tables so
    gather indices fit in int16 (dma_gather requirement).
  - Device (per core): dma_gather edge source rows (bf16, 256B rows) in
    groups, emitted eagerly in consumption order and spread over all 4 SWDGE
    queues so descriptor generation and the SDMA drain overlap deeply.  Per
    (slot, stream) segment one broadcast tensor_tensor builds all one-hot
    chunks at once (bf16, never enters DVE 2-port mode so it cannot stall
    SWDGE desc-gen); per 128-edge chunk one matmul accumulates msgs.T @
    onehot into PSUM, yielding agg.T per node tile; then out.T = W.T @ agg.T
    on the TensorEngine and a fused bias-add on the Scalar engine; DMA out.T
    tiles to DRAM.
  - Host: transpose + scatter per-core tile outputs back to [50000, 128].
"""

import os
import sys

for _p in ("/opt/trn_rl_repo",):
    if _p not in sys.path and os.path.isdir(_p):
        sys.path.insert(0, _p)

import numpy as np
import ml_dtypes

P = 128
N_NODES = 50000
N_EDGES = 640000
D = 128
NCORES = 8
HALF = 25000          # int16 index-range split of the feature table
NTILE = (N_NODES + P - 1) // P          # 391
NSLOT = (NTILE + NCORES - 1) // NCORES  # 49 node tiles per core
GCHUNK = 16           # chunks (of 128 gathered rows) per dma_gather call
NQUEUES = 4           # SWDGE queues; gather desc-gen contexts run concurrently
GBUFS = 8             # in-flight gather buffers per stream
SINGLE_PACKET = False

BF16 = ml_dtypes.bfloat16


# ---------------------------------------------------------------- host plan

def plan(src, dst):
    """Pack nodes into custom (core, slot) tiles of <=128 nodes so that each
    tile's lo/hi edge counts land just under chunk (128-edge) boundaries and
    cores are balanced; then lay out padded, chunked edge lists.  Chunk
    counts are shared across cores (max over cores) so the single SPMD
    program fits every core."""
    src = np.asarray(src).astype(np.int64)
    dst = np.asarray(dst).astype(np.int64)
    d_lo = np.bincount(dst[src < HALF], minlength=N_NODES).astype(np.int64)
    d_hi = np.bincount(dst[src >= HALF], minlength=N_NODES).astype(np.int64)
    d = d_lo + d_hi

    # Phase 1: nodes -> cores (LPT on total degree, node-count cap)
    order = np.argsort(-d, kind="stable")
    core_tot = np.zeros(NCORES)
    core_n = np.zeros(NCORES, int)
    core_of = np.empty(N_NODES, int)
    cap = NSLOT * P
    for n in order:
        c = min((c for c in range(NCORES) if core_n[c] < cap),
                key=lambda c: core_tot[c])
        core_of[n] = c
        core_tot[c] += d[n]
        core_n[c] += 1

    # Phase 2: common per-slot chunk schedule (shared across cores)
    SLACK = 3
    lo_tot = np.bincount(core_of, weights=d_lo, minlength=NCORES)
    hi_tot = np.bincount(core_of, weights=d_hi, minlength=NCORES)
    KA = int(np.ceil(lo_tot.max() / P)) + SLACK
    KB = int(np.ceil(hi_tot.max() / P)) + SLACK

    def distribute(K, slots):
        base, extra = K // slots, K - (K // slots) * slots
        return np.array([base + 1] * extra + [base] * (slots - extra))

    A = distribute(KA, NSLOT)
    B = distribute(KB, NSLOT)[::-1]

    # Phase 3: per core, fill slots steering (lo, hi) sums to the caps
    node_lists = [[None] * NSLOT for _ in range(NCORES)]
    Klo_all = np.zeros((NCORES, NSLOT), int)
    Khi_all = np.zeros((NCORES, NSLOT), int)
    slot_of = np.empty(N_NODES, int)
    pos_of = np.empty(N_NODES, int)
    for c in range(NCORES):
        nodes = np.where(core_of == c)[0]
        dl = d_lo[nodes].astype(np.float64)
        dh = d_hi[nodes].astype(np.float64)
        alive = np.ones(len(nodes), bool)
        for s in range(NSLOT):
            TL, TH = A[s] * P, B[s] * P
            lo = hi = 0.0
            rn = P
            take = []
            idxs = np.where(alive)[0]
            for _ in range(min(P, int(alive.sum()))):
                idxs = idxs[alive[idxs]]
                if len(idxs) == 0:
                    break
                bl, bh = (TL - lo) / rn, (TH - hi) / rn
                ok = (dl[idxs] <= TL - lo) & (dh[idxs] <= TH - hi)
                cand = idxs[ok] if ok.any() else idxs
                pick = cand[np.argmin(np.abs(dl[cand] - bl) +
                                      np.abs(dh[cand] - bh))]
                take.append(pick)
                alive[pick] = False
                lo += dl[pick]
                hi += dh[pick]
                rn -= 1
                if rn == 0:
                    break
            ns = nodes[np.array(take, int)] if take else np.empty(0, int)
            node_lists[c][s] = ns
            slot_of[ns] = s
            pos_of[ns] = np.arange(len(ns))
            Klo_all[c, s] = -(-int(lo) // P)
            Khi_all[c, s] = -(-int(hi) // P)

    Klo = Klo_all.max(axis=0)
    Khi = Khi_all.max(axis=0)
    for s in range(NSLOT):  # every slot needs >=1 chunk so PSUM is written
        if Klo[s] + Khi[s] == 0:
            Klo[s] = 1
    KLO, KHI = int(Klo.sum()), int(Khi.sum())

    lo_base = np.concatenate([[0], np.cumsum(Klo)])[:NSLOT]
    hi_base = np.concatenate([[0], np.cumsum(Khi)])[:NSLOT]

    # edges grouped by (core, slot)
    ekey = core_of[dst] * NSLOT + slot_of[dst]
    edge_order = np.argsort(ekey, kind="stable")
    ekey_sorted = ekey[edge_order]
    starts = np.searchsorted(ekey_sorted, np.arange(NCORES * NSLOT))
    ends = np.searchsorted(ekey_sorted, np.arange(NCORES * NSLOT), side="right")

    # per-core padded index / dst_local arrays, chunk-major [K, 128]
    idx = np.zeros((NCORES, KLO + KHI, P), np.int16)
    dstl = np.full((NCORES, KLO + KHI, P), -1.0, np.float32)
    for c in range(NCORES):
        for s in range(NSLOT):
            e_all = edge_order[starts[c * NSLOT + s]:ends[c * NSLOT + s]]
            m = src[e_all] < HALF
            for K, bases, e, stream_off, table_off in (
                (Klo[s], lo_base, e_all[m], 0, 0),
                (Khi[s], hi_base, e_all[~m], KLO, HALF),
            ):
                if K == 0:
                    continue
                b0 = stream_off + bases[s]
                flat_i = idx[c, b0:b0 + K].reshape(-1)
                flat_d = dstl[c, b0:b0 + K].reshape(-1)
                flat_i[: len(e)] = (src[e] - table_off).astype(np.int16)
                flat_d[: len(e)] = pos_of[dst[e]].astype(np.float32)

    return {
        "node_lists": node_lists,
        "Klo": Klo, "Khi": Khi, "KLO": KLO, "KHI": KHI,
        "idx": idx, "dstl": dstl,
    }


def _groups(K):
    """Split stream of K chunks into gather groups of <= GCHUNK chunks."""
    out = []
    c = 0
    while c < K:
        out.append((c, min(c + GCHUNK, K)))
        c = out[-1][1]
    return out


def _interleave(Klo, Khi):
    """Order lo/hi gather groups by first consumption point: walk the slots
    (lo segment then hi segment per slot) and append a stream's next group
    when the consumption cursor first enters it."""
    lo_groups, hi_groups = _groups(int(np.sum(Klo))), _groups(int(np.sum(Khi)))
    order = []
    nxt = {"lo": 0, "hi": 0}
    cur = {"lo": 0, "hi": 0}
    groups = {"lo": lo_groups, "hi": hi_groups}
    for s in range(len(Klo)):
        for name, k in (("lo", int(Klo[s])), ("hi", int(Khi[s]))):
            cur[name] += k
            while nxt[name] < len(groups[name]) and \
                    groups[name][nxt[name]][0] < cur[name]:
                order.append((name, groups[name][nxt[name]]))
                nxt[name] += 1
    return lo_groups, hi_groups, order


def pack_gidx(idx):
    """[K,128] int16 chunk-major indices -> [128, K*8] dma_gather layout
    (index i of a group at [i%16, i//16], replicated on partitions 16..127)."""
    K = idx.shape[0]
    out = np.zeros((128, K * 8), np.int16)
    for c0, c1 in _groups(K):
        g = idx[c0:c1].reshape(-1)                # i = (c-c0)*128 + lane
        blk = g.reshape(-1, 16).T                 # [16, (c1-c0)*8]
        out[:, c0 * 8:c1 * 8] = np.tile(blk, (8, 1))
    return out


# ---------------------------------------------------------------- program

def build(Klo, Khi, dbg=False):
    import concourse.bass as bass
    import concourse.mybir as mybir
    from concourse import bacc
    import concourse.tile as tile

    KLO, KHI = int(np.sum(Klo)), int(np.sum(Khi))
    NCH = KLO + KHI
    bf16, f32, i16 = mybir.dt.bfloat16, mybir.dt.float32, mybir.dt.int16

    lo_base = np.concatenate([[0], np.cumsum(Klo)])[:NSLOT]
    hi_base = np.concatenate([[0], np.cumsum(Khi)])[:NSLOT]

    nc = bacc.Bacc("TRN2", debug=dbg, num_swdge_queues=NQUEUES)
    flo = nc.dram_tensor("flo", [HALF, D], bf16, kind="ExternalInput")
    fhi = nc.dram_tensor("fhi", [N_NODES - HALF, D], bf16, kind="ExternalInput")
    gidx = nc.dram_tensor("gidx", [P, NCH * 8], i16, kind="ExternalInput")
    dstl = nc.dram_tensor("dstl", [P, NCH], bf16, kind="ExternalInput")
    iota = nc.dram_tensor("iota", [P, P], bf16, kind="ExternalInput")
    wmat = nc.dram_tensor("wmat", [P, P], bf16, kind="ExternalInput")
    bcol = nc.dram_tensor("bcol", [P, 1], f32, kind="ExternalInput")
    out = nc.dram_tensor("out", [P, NSLOT * P], f32, kind="ExternalOutput")

    lo_groups, hi_groups, gorder = _interleave(Klo, Khi)
    KSEG = int(max(Klo.max(), Khi.max()))

    with tile.TileContext(nc) as tc:
        with tc.tile_pool(name="const", bufs=1) as cp, \
             tc.tile_pool(name="glo", bufs=GBUFS) as gplo, \
             tc.tile_pool(name="ghi", bufs=GBUFS) as gphi, \
             tc.tile_pool(name="oh", bufs=6) as ohp, \
             tc.tile_pool(name="res", bufs=3) as resp, \
             tc.tile_pool(name="psA", bufs=4, space="PSUM") as psA, \
             tc.tile_pool(name="psB", bufs=2, space="PSUM") as psB:

            # gather indices for the first two groups land first so the first
            # gathers can issue immediately; everything else loads behind them
            gidx_t = cp.tile([P, NCH * 8], i16)
            first = gorder[:2]
            done = {"lo": 0, "hi": 0}
            for name, (c0, c1) in first:
                off = 0 if name == "lo" else KLO
                nc.sync.dma_start(out=gidx_t[:, (off + c0) * 8:(off + c1) * 8],
                                  in_=gidx[:, (off + c0) * 8:(off + c1) * 8])
                done[name] = max(done[name], c1)

            st = {
                "lo": {"groups": lo_groups, "tab": flo, "pool": gplo,
                       "tiles": {}, "coff": 0, "g": 0},
                "hi": {"groups": hi_groups, "tab": fhi, "pool": gphi,
                       "tiles": {}, "coff": KLO, "g": 0},
            }

            qcount = [0]

            def fetch(name):
                S = st[name]
                gi = S["g"]
                c0, c1 = S["groups"][gi]
                n = c1 - c0
                t = S["pool"].tile([P, n * P], mybir.dt.bfloat16, tag="g" + name)
                nc.gpsimd.dma_gather(
                    out_ap=t[:].rearrange("p (g d) -> p g d", d=P),
                    in_ap=S["tab"][:],
                    idxs_ap=gidx_t[:, (S["coff"] + c0) * 8:(S["coff"] + c1) * 8],
                    num_idxs=n * P,
                    num_idxs_reg=n * P,
                    elem_size=P,
                    single_packet=SINGLE_PACKET,
                    queue_num=qcount[0] % NQUEUES,
                )
                qcount[0] += 1
                S["tiles"][gi] = (t, c0, c1)
                S["g"] += 1

            # first gathers, then the rest of the constants, then all other
            # gathers (consumption order; buffer pool depth throttles them)
            for name, _ in first:
                fetch(name)

            # remaining gidx in two bulk DMAs (lo tail, hi tail)
            for name, K in (("lo", KLO), ("hi", KHI)):
                off = 0 if name == "lo" else KLO
                c0 = done[name]
                if c0 < K:
                    nc.sync.dma_start(out=gidx_t[:, (off + c0) * 8:(off + K) * 8],
                                      in_=gidx[:, (off + c0) * 8:(off + K) * 8])
            dstl_t = cp.tile([P, NCH], bf16)
            nc.sync.dma_start(out=dstl_t[:], in_=dstl[:])
            iota_t = cp.tile([P, P], bf16)
            nc.sync.dma_start(out=iota_t[:], in_=iota[:])
            w_t = cp.tile([P, P], bf16)
            nc.sync.dma_start(out=w_t[:], in_=wmat[:])
            b_t = cp.tile([P, 1], f32)
            nc.sync.dma_start(out=b_t[:], in_=bcol[:])

            for name, _ in gorder[2:]:
                fetch(name)

            def onehot(dc0, k):
                """one tensor_tensor -> [P, k*128] bf16 one-hot for k chunks
                whose dstl columns start at global chunk dc0."""
                oh = ohp.tile([P, KSEG * P], mybir.dt.bfloat16, tag="oh")
                in0 = iota_t[:].rearrange("p (k f) -> p k f", k=1) \
                    .broadcast_to([P, k, P])
                in1 = dstl_t[:, dc0:dc0 + k].rearrange("p (k o) -> p k o", o=1) \
                    .broadcast_to([P, k, P])
                outv = oh[:, :k * P].rearrange("p (k f) -> p k f", k=k)
                nc.vector.tensor_tensor(out=outv, in0=in0, in1=in1,
                                        op=mybir.AluOpType.is_equal)
                return oh

            cur = {"lo": 0, "hi": 0}  # global chunk cursor per stream
            gcur = {"lo": 0, "hi": 0}  # current group idx per stream
            for s in range(NSLOT):
                nch = int(Klo[s] + Khi[s])
                ps_agg = psA.tile([P, P], f32, tag="agg")
                ci = 0
                for name, k, sbase in (("lo", int(Klo[s]), int(lo_base[s])),
                                       ("hi", int(Khi[s]), int(hi_base[s]))):
                    if k == 0:
                        continue
                    S = st[name]
                    oh = onehot(S["coff"] + sbase, k)
                    for j in range(k):
                        pos = cur[name]
                        while pos >= S["groups"][gcur[name]][1]:
                            gcur[name] += 1
                        t, c0, c1 = S["tiles"][gcur[name]]
                        off = pos - c0
                        nc.tensor.matmul(
                            out=ps_agg[:],
                            lhsT=t[:, off * P:(off + 1) * P],
                            rhs=oh[:, j * P:(j + 1) * P],
                            start=(ci == 0), stop=(ci == nch - 1),
                        )
                        cur[name] += 1
                        ci += 1

                aggT = resp.tile([P, P], mybir.dt.bfloat16, tag="aggT")
                nc.scalar.copy(out=aggT[:], in_=ps_agg[:])
                ps_out = psB.tile([P, P], f32, tag="out")
                nc.tensor.matmul(out=ps_out[:], lhsT=w_t[:], rhs=aggT[:],
                                 start=True, stop=True)
                o_sb = resp.tile([P, P], f32, tag="osb")
                nc.scalar.activation(
                    out=o_sb[:], in_=ps_out[:],
                    func=mybir.ActivationFunctionType.Identity,
                    bias=b_t[:, 0:1],
                )
                nc.sync.dma_start(out=out[:, s * P:(s + 1) * P], in_=o_sb[:])

    # Spread gathers across SWDGE queues.  Tile assigns each Pool-engine DMA
    # a DMASW completion lane in *scheduled* order; queue choice must be a
    # function of that lane (the sim/ucode bind each lane to one queue), so
    # retag after scheduling: queue = lane % NQUEUES.
    for inst in nc.inst_map.values():
        if isinstance(inst, mybir.InstDMAGatherAnt):
            proc = inst.bass_scheduled_proc
            if proc is not None and 11 <= proc <= 18:
                inst.queue_num = (proc - 11) % NQUEUES

    nc.compile()
    return nc


# ---------------------------------------------------------------- in_maps

def make_in_maps(features, W, b, pl):
    f16 = np.ascontiguousarray(features).astype(BF16)
    iota_np = np.tile(np.arange(P, dtype=np.float32)[None, :], (P, 1)).astype(BF16)
    w_np = np.asarray(W, np.float32).astype(BF16)
    b_np = np.asarray(b, np.float32).reshape(1, D).T.copy()  # [128,1]
    in_maps = []
    for c in range(NCORES):
        in_maps.append({
            "flo": f16[:HALF],
            "fhi": f16[HALF:],
            "gidx": pack_gidx(pl["idx"][c]),
            "dstl": np.ascontiguousarray(pl["dstl"][c].T).astype(BF16),
            "iota": iota_np,
            "wmat": w_np,
            "bcol": b_np,
        })
    return in_maps


def unshard(outs, node_lists):
    """outs: list of {'out': [128, NSLOT*128] f32} per core -> [50000,128]."""
    full = np.zeros((N_NODES, D), np.float32)
    for c in range(NCORES):
        oT = np.asarray(outs[c]["out"], np.float32)
        for s in range(NSLOT):
            ns = node_lists[c][s]
            if len(ns) == 0:
                continue
            full[ns, :] = oT[:, s * P:s * P + len(ns)].T
    return full


# ---------------------------------------------------------------- entry

_CACHE = {}


def kernel(features, src, dst, W, b):
    from concourse.bass_utils import run_bass_kernel_spmd

    pl = plan(src, dst)
    key = (tuple(pl["Klo"]), tuple(pl["Khi"]))
    if key not in _CACHE:
        _CACHE[key] = build(pl["Klo"], pl["Khi"])
    nc = _CACHE[key]
    in_maps = make_in_maps(features, W, b, pl)
    last = None
    for _ in range(3):  # retry: a previously wedged pool device can fail a load
        try:
            res = run_bass_kernel_spmd(nc, in_maps, core_ids=list(range(NCORES)))
            return unshard(res.results, pl["node_lists"])
        except Exception as e:  # noqa: BLE001
            last = e
    raise last
